# revision 8
# baseline (speedup 1.0000x reference)
"""Grouped-experts SwiGLU MoE kernel for 8 Trainium2 NeuronCores.

Expert-parallel: core i owns expert i (E == n_cores == 8) and the 2048
tokens routed to it (tokens are pre-sorted by expert, even split).

Per-core math (Ti=2048, D=2048, H=5632):
    gate = x_i @ w1_i.T ; up = x_i @ w3_i.T
    h    = silu(gate) * up
    out  = h @ w2_i.T

Device layout strategy: the TensorE matmul contracts over the partition
dim, so every operand is staged with the contraction dim on partitions.
All tensors are pre-transposed/cast on the host so every device DMA is a
natural contiguous load:
    xT  (D, Ti)  fp16   GEMM1/3 moving operand
    w1T (D, H)   fp16   GEMM1 stationary operand
    w3T (D, H)   fp16   GEMM3 stationary operand
    w2T (H, D)   fp16   GEMM2 moving operand
Matmuls run in fp16 (full PE rate) with fp32 PSUM accumulation; the
SwiGLU intermediate h is produced transposed (hT, [H, T] tiles, fp16 in
SBUF) and consumed directly as the GEMM2 stationary operand, so no
on-device transposes are needed anywhere.

Both passes use 512-wide moving operands (one full PSUM bank), the
maximum per-matmul tile, so per-instruction overhead is minimal.  The
x tile is DMA'd in four contraction slices, interleaved with the first
weight chunk, so the PE can start within a few us of kernel entry
instead of waiting for 4 MB of serial DMA.
"""

import numpy as np

import concourse.tile as tile
import concourse.mybir as mybir
from concourse import bacc
from concourse.bass_utils import run_bass_kernel_spmd

E, T, D, H = 8, 16384, 2048, 5632
NCORES = 8
TCORE = T // NCORES

_prog_cache: dict = {}


def _build_program(TCORE_=TCORE, D_=D, H_=H, TB=512, HC=256, DB=512, loop_n=1,
                   staggered=False, hints=False, body_reps=1, pg_bufs=2):
    """Build + schedule + compile the per-core Bass program (SPMD).

    loop_n > 1 wraps the whole body in a hardware loop that recomputes
    the identical result loop_n times — used only for timing runs.
    staggered (timing runs only) uses the staggered-reset loop back-edge
    with one pipeline stage per token block, so iteration boundaries
    overlap instead of paying a full all-engine barrier.
    """
    f16 = mybir.dt.float16
    f32 = mybir.dt.float32
    P = 128
    DO = D_ // P            # contraction subtiles for GEMM1/3
    HO = H_ // P            # contraction subtiles for GEMM2
    NTB = TCORE_ // TB      # token blocks
    NHC = H_ // HC          # h chunks in pass A
    NHS = HC // P           # h subtiles per chunk
    NDB = D_ // DB          # d chunks in pass B
    NTT = TB // P           # t tiles in pass B
    assert D_ % P == 0 and H_ % HC == 0 and HC % P == 0
    assert TCORE_ % TB == 0 and D_ % DB == 0 and TB % P == 0
    assert not staggered or (loop_n > 1 and NTB == 4)

    nc = bacc.Bacc(None, target_bir_lowering=False)
    xT = nc.dram_tensor("xT", [D_, TCORE_], f16, kind="ExternalInput")
    w1T = nc.dram_tensor("w1T", [D_, H_], f16, kind="ExternalInput")
    w3T = nc.dram_tensor("w3T", [D_, H_], f16, kind="ExternalInput")
    w2T = nc.dram_tensor("w2T", [H_, D_], f16, kind="ExternalInput")
    out = nc.dram_tensor("out", [TCORE_, D_], f32, kind="ExternalOutput")

    xTr = xT[:].rearrange("(do p) t -> p do t", p=P)
    w1Tr = w1T[:].rearrange("(do p) h -> p do h", p=P)
    w3Tr = w3T[:].rearrange("(do p) h -> p do h", p=P)
    w2Tr = w2T[:].rearrange("(ho p) d -> p ho d", p=P)

    silu = mybir.ActivationFunctionType.Silu

    with tile.TileContext(nc) as tc:
        with (
            tc.tile_pool(name="xpool", bufs=1) as xpool,
            tc.tile_pool(name="wpool", bufs=2) as wpool,
            tc.tile_pool(name="hpool", bufs=1) as hpool,
            tc.tile_pool(name="w2pool", bufs=2) as w2pool,
            tc.tile_pool(name="spool", bufs=3) as spool,
            tc.tile_pool(name="opool", bufs=3) as opool,
            tc.tile_pool(name="pgate", bufs=pg_bufs, space="PSUM") as pgate,
            tc.tile_pool(name="pout", bufs=2, space="PSUM") as pout,
        ):
            import contextlib
            if loop_n > 1:
                eng = mybir.EngineType
                loop_ctx = tc.For_i(
                    0, loop_n, 1,
                    staggered_reset=staggered,
                    hint_engines=(eng.PE, eng.SP, eng.DVE, eng.Activation,
                                  eng.Pool) if (staggered or hints) else (),
                )
            else:
                loop_ctx = contextlib.nullcontext()
            with loop_ctx:
                for _rep in range(body_reps):
                    _emit_body(
                        nc, tc, xpool, wpool, hpool, w2pool, spool, opool,
                        pgate, pout, xTr, w1Tr, w3Tr, w2Tr, out,
                        P, DO, HO, NTB, NHC, NHS, NDB, NTT, TB, HC, DB, silu,
                        staggered=staggered,
                    )
    nc.compile()
    return nc


def _emit_body(nc, tc, xpool, wpool, hpool, w2pool, spool, opool,
               pgate, pout, xTr, w1Tr, w3Tr, w2Tr, out,
               P, DO, HO, NTB, NHC, NHS, NDB, NTT, TB, HC, DB, silu,
               staggered=False):
    f16 = mybir.dt.float16
    f32 = mybir.dt.float32
    XS = 4                      # x DMA split along the contraction dim
    assert DO % XS == 0
    DOS = DO // XS
    for tb in range(NTB):
        if staggered and tb > 0:
            tc.stage_boundary()
        tsl_all = slice(tb * TB, (tb + 1) * TB)
        x_t = xpool.tile([P, DO, TB], f16, tag="xt")

        def load_x(piece):
            dsl = slice(piece * DOS, (piece + 1) * DOS)
            nc.sync.dma_start(x_t[:, dsl, :], xTr[:, dsl, tsl_all])

        # First w chunk interleaved with the x slices so the first
        # matmul group can start after ~2 small DMAs, not 4 MB.
        load_x(0)
        w1t0 = wpool.tile([P, DO, HC], f16, tag="w1t")
        nc.sync.dma_start(w1t0[:], w1Tr[:, :, 0:HC])
        for piece in range(1, XS):
            load_x(piece)
        w3t0 = wpool.tile([P, DO, HC], f16, tag="w3t")
        nc.sync.dma_start(w3t0[:], w3Tr[:, :, 0:HC])

        hT = hpool.tile([P, HO, TB], f16, tag="ht")

        # ---- pass A: hT[h, t] = silu(w1T.T x) * (w3T.T x) ----
        for hc in range(NHC):
            hsl_all = slice(hc * HC, (hc + 1) * HC)
            if hc == 0:
                w1t, w3t = w1t0, w3t0
            else:
                w1t = wpool.tile([P, DO, HC], f16, tag="w1t")
                nc.sync.dma_start(w1t[:], w1Tr[:, :, hsl_all])
                w3t = wpool.tile([P, DO, HC], f16, tag="w3t")
                nc.sync.dma_start(w3t[:], w3Tr[:, :, hsl_all])
            for hs in range(NHS):
                hsl = slice(hs * P, (hs + 1) * P)
                gate = pgate.tile([P, TB], f32, tag="gate")
                up = pgate.tile([P, TB], f32, tag="up")
                for do in range(DO):
                    nc.tensor.matmul(
                        gate[:], w1t[:, do, hsl], x_t[:, do, :],
                        start=(do == 0), stop=(do == DO - 1),
                    )
                for do in range(DO):
                    nc.tensor.matmul(
                        up[:], w3t[:, do, hsl], x_t[:, do, :],
                        start=(do == 0), stop=(do == DO - 1),
                    )
                sil = spool.tile([P, TB], f32, tag="sil")
                nc.scalar.activation(sil[:], gate[:], silu)
                nc.vector.tensor_mul(
                    hT[:, hc * NHS + hs, :], sil[:], up[:]
                )

        # ---- pass B: out[t, d] = hT.T @ w2T ----
        for db in range(NDB):
            dsl = slice(db * DB, (db + 1) * DB)
            w2t = w2pool.tile([P, HO, DB], f16, tag="w2t")
            nc.sync.dma_start(w2t[:], w2Tr[:, :, dsl])
            for tt in range(NTT):
                tsl = slice(tt * P, (tt + 1) * P)
                ops = pout.tile([P, DB], f32, tag="ops")
                for ho in range(HO):
                    nc.tensor.matmul(
                        ops[:], hT[:, ho, tsl], w2t[:, ho, :],
                        start=(ho == 0), stop=(ho == HO - 1),
                    )
                ob = opool.tile([P, DB], f32, tag="ob")
                nc.vector.tensor_copy(ob[:], ops[:])
                nc.sync.dma_start(
                    out[tb * TB + tt * P: tb * TB + (tt + 1) * P, dsl],
                    ob[:],
                )


def _get_program(**kw):
    key = tuple(sorted(kw.items()))
    if key not in _prog_cache:
        _prog_cache[key] = _build_program(**kw)
    return _prog_cache[key]


def _host_prep(x_i, w1_i, w2_i, w3_i):
    return {
        "xT": np.ascontiguousarray(x_i.T, dtype=np.float16),
        "w1T": np.ascontiguousarray(w1_i.T, dtype=np.float16),
        "w3T": np.ascontiguousarray(w3_i.T, dtype=np.float16),
        "w2T": np.ascontiguousarray(w2_i.T, dtype=np.float16),
    }


def _numpy_fallback(x, w1, w2, w3, counts):
    outs = []
    start = 0
    for e in range(len(counts)):
        n = int(counts[e])
        xe = x[start:start + n]
        gate = xe @ w1[e].T
        up = xe @ w3[e].T
        h = (gate / (1.0 + np.exp(-gate))) * up
        outs.append(h @ w2[e].T)
        start += n
    return np.concatenate(outs, axis=0).astype(np.float32)


def kernel(x, w1, w2, w3, num_tokens_per_expert, _trace=False):
    x = np.asarray(x, dtype=np.float32)
    w1 = np.asarray(w1, dtype=np.float32)
    w2 = np.asarray(w2, dtype=np.float32)
    w3 = np.asarray(w3, dtype=np.float32)
    counts = np.asarray(num_tokens_per_expert).astype(np.int64)

    if not (len(counts) == E and np.all(counts == TCORE)):
        return _numpy_fallback(x, w1, w2, w3, counts)

    nc = _get_program()
    in_maps = [
        _host_prep(x[i * TCORE:(i + 1) * TCORE], w1[i], w2[i], w3[i])
        for i in range(NCORES)
    ]
    res = run_bass_kernel_spmd(
        nc, in_maps, core_ids=list(range(NCORES)), trace=_trace
    )
    out = np.concatenate([r["out"] for r in res.results], axis=0)
    if _trace:
        return out, res
    return out
